# revision 20
# baseline (speedup 1.0000x reference)
"""Trainium2 Bass kernel for nn_CLF_block (channel-attention block).

Reference computation (per batch item b, with x = concat([a,b], ch) in [256, N],
N = H*W = 16384):
    z  = w1 x + b1 1^T
    q  = w2 z + b2 1^T ;  k = w3 z + b3 1^T ;  v = w4 z + b4 1^T
    qk = q k^T ; attn = softmax(qk, -1) ; out = attn v

Algebraic restructuring (two O(C^2 N) passes over x, everything else is
256x256 algebra):
    Gx = x x^T ; sx = x 1                      (one fp16 pass over x)
    u  = w1 sx ; s = u + N b1
    G  = w1 Gx w1^T + u b1^T + b1 u^T + N b1 b1^T        (= z z^T)
    qk = w2 G w3^T + (w2 s) b3^T + b2 (w3 s)^T + N b2 b3^T
    attn = softmax(qk)
    W  = attn (w4 w1) ; c0 = attn (w4 b1 + b4)  (w4 w1, w4 b1 + b4 from host)
    out = W x + c0 1^T                          (second fp16 pass over x)

x is downloaded ONCE as fp16 in natural layout (8 MiB/core); the gram pass
consumes on-chip PE transposes of it (Gx symmetry nearly halves the gram
work: block (1,0) is recovered as block (0,1)^T).  The 256x256 algebra runs
in fp32 on the PE with diagonal-splitting at the Gx and G levels (the same
scheme the reference two-download kernel used; hardware fp32r casts on the
DVE proved lossy, so no on-chip f32r is used).  The W = attn(w4 w1) product
runs in fp16.  Pass 2 is fp16 x fp16 with fine-grained PSUM units and the
output is written fp16 (8 MiB/core), upcast on the host.

HBM per core: ~9.8 MiB in + 8 MiB out (vs ~48 MiB for the two-download
scheme) -> ~118 us vs 217 us baseline.

Sharding: data-parallel over batch, one batch item per NeuronCore (B=8, 8 cores).
"""

import sys

if "/opt/trn_rl_repo" not in sys.path:
    sys.path.insert(0, "/opt/trn_rl_repo")

from contextlib import ExitStack

import numpy as np

import concourse.bass as bass
import concourse.mybir as mybir
import concourse.tile as tile
from concourse import bacc
from concourse.bass_utils import run_bass_kernel_spmd

F32 = mybir.dt.float32
F32R = mybir.dt.float32r
F16 = mybir.dt.float16
P = 128           # partitions / channel block
C = 256           # channels
NPIX = 128 * 128  # spatial positions per batch item
NPIECE = 4        # x DMA pieces per channel half
PIECE = NPIX // NPIECE   # 4096 cols per piece
NCHUNK = NPIX // P       # 128 gram chunks
GRP = 8           # chunks per transpose group (one PSUM bank = 8 x [128,128] fp16)
NGRP = NCHUNK // GRP     # 16 groups
NT = 512          # matmul moving-operand width for pass 2


def _emit(nc, tc, ctx, d_in, d_out):
    """Emit the Tile program for one core (one batch item)."""
    wcat, ident, identh = d_in["wcat"], d_in["ident"], d_in["identh"]
    x0_d, x1_d = d_in["x0"], d_in["x1"]
    brows, bcols = d_in["brows"], d_in["bcols"]
    out_d = d_out["out"]

    const = ctx.enter_context(tc.tile_pool(name="const", bufs=1))
    xpool = ctx.enter_context(tc.tile_pool(name="xpool", bufs=1))

    # --- small constants (head of the scalar queue, ~0.4 MiB) ------------
    identh_sb = const.tile([P, P], F16, name="identh_sb", tag="identh_sb")
    nc.sync.dma_start(out=identh_sb, in_=identh[:, :])

    rows = []
    for r in range(3):
        rt = const.tile([1, C], F32, name=f"brow{r}", tag=f"brow{r}")
        nc.scalar.dma_start(out=rt, in_=brows[r:r + 1, :])
        rows.append(rt)
    rho2_row, rho3_row, nrho3_row = rows

    bc_sb = []
    for k in range(2):
        bt = const.tile([P, 4], F32, name=f"bcol{k}", tag=f"bcol{k}")
        nc.scalar.dma_start(out=bt, in_=bcols[k * P:(k + 1) * P, :])
        bc_sb.append(bt)
    b1_col = [bc_sb[k][:, 0:1] for k in range(2)]
    nb1_col = [bc_sb[k][:, 1:2] for k in range(2)]
    w4b_col = [bc_sb[k][:, 2:3] for k in range(2)]

    eh = d_in["eh"]
    eh_sb = []
    for k in range(2):
        et = const.tile([P, C], F16, name=f"eh_sb{k}", tag=f"eh_sb{k}")
        nc.scalar.dma_start(out=et, in_=eh[k * P:(k + 1) * P, :])
        eh_sb.append(et)

    ident_sb = const.tile([P, P], F32R, name="ident_sb", tag="ident_sb")
    nc.scalar.dma_start(out=ident_sb, in_=ident[:, :])

    # --- resident fp16 x in natural layout.  All x pieces go on the sync
    # queue (scalar must stay free for phase-1 staging copies); the first
    # piece is split into 1024-col sub-DMAs so the PE can start early.
    xs = [[None] * NPIECE, [None] * NPIECE]
    for c in range(2):
        for i in range(NPIECE):
            xs[c][i] = xpool.tile([P, PIECE], F16, name=f"x{c}_{i}",
                                  tag=f"x{c}_{i}")
    subs = [(0, 512), (512, 512), (1024, 1024), (2048, 1024), (3072, 1024)]
    for o, w in subs:
        for c, src_d in ((0, x0_d), (1, x1_d)):
            nc.sync.dma_start(out=xs[c][0][:, o:o + w],
                              in_=src_d[:, o:o + w])
    for i in range(1, NPIECE):
        for c, src_d in ((0, x0_d), (1, x1_d)):
            nc.sync.dma_start(out=xs[c][i],
                              in_=src_d[:, i * PIECE:(i + 1) * PIECE])

    # --- big weights last on sync (needed only after the gram) -----------
    w_sb = []
    for k in range(2):
        wt = const.tile([P, 2 * C], F32, name=f"w_sb{k}", tag=f"w_sb{k}")
        nc.sync.dma_start(out=wt, in_=wcat[k * P:(k + 1) * P, :])
        w_sb.append(wt)
    At = [w_sb[k][:, 0 * C:1 * C] for k in range(2)]    # (w2 w1)^T  [c', o]
    Bt = [w_sb[k][:, 1 * C:2 * C] for k in range(2)]    # (w3 w1)^T  [c', o]

    # --- pass 1: Gx = x x^T via on-chip PE transposes --------------------
    # Per 128-col chunk: transpose both channel-half chunks into a PSUM
    # bank (8 chunks batched), copy to an ones-augmented fp16 staging tile,
    # then two accumulating gram matmuls.  Symmetry: only blocks (0,*) and
    # (1,1) are computed; block (1,0) = block (0,1)^T afterwards.
    stg = []
    for s in range(2):
        st = const.tile([P, GRP, C + 1], F16, name=f"stg{s}", tag=f"stg{s}")
        nc.vector.memset(st[:, :, C:C + 1], 1.0)
        stg.append(st)

    gx_sb = [
        const.tile([P, C + 1], F32, name=f"gx_sb{b}", tag=f"gx_sb{b}")
        for b in range(2)
    ]

    CH_PP = PIECE // P  # chunks per piece
    with tc.tile_pool(name="gx_ps", bufs=1, space="PSUM") as gxp, \
         tc.tile_pool(name="tp_ps", bufs=1, space="PSUM") as tpp:
        shh0 = gxp.tile([P, C + 1], F32, name="shh0", tag="shh0")
        shh1 = gxp.tile([P, P + 1], F32, name="shh1", tag="shh1")

        def emit_transposes(grp):
            s = grp % 2
            tpA = tpp.tile([P, GRP, P], F16, name="tpA", tag=f"tpA{s}")
            tpB = tpp.tile([P, GRP, P], F16, name="tpB", tag=f"tpB{s}")
            for g in range(GRP):
                ch = grp * GRP + g
                i, lc = divmod(ch, CH_PP)
                nc.tensor.transpose(tpA[:, g, :],
                                    xs[0][i][:, lc * P:(lc + 1) * P],
                                    identh_sb)
                nc.tensor.transpose(tpB[:, g, :],
                                    xs[1][i][:, lc * P:(lc + 1) * P],
                                    identh_sb)
            nc.vector.tensor_copy(stg[s][:, :, 0:P], tpA)
            nc.scalar.activation(out=stg[s][:, :, P:C], in_=tpB,
                                 func=mybir.ActivationFunctionType.Identity,
                                 scale=1.0)

        def emit_grams(grp):
            s = grp % 2
            for g in range(GRP):
                ch = grp * GRP + g
                nc.tensor.matmul(shh0, stg[s][:, g, 0:P],
                                 stg[s][:, g, 0:C + 1],
                                 start=(ch == 0), stop=(ch == NCHUNK - 1))
                nc.tensor.matmul(shh1, stg[s][:, g, P:C],
                                 stg[s][:, g, P:C + 1],
                                 start=(ch == 0), stop=(ch == NCHUNK - 1))

        for grp in range(NGRP + 1):
            if grp < NGRP:
                emit_transposes(grp)
            if grp >= 1:
                emit_grams(grp - 1)

        # Assemble full Gx (with sx in col 256) from the symmetric pieces.
        nc.vector.tensor_copy(gx_sb[0], shh0)
        nc.vector.tensor_copy(gx_sb[1][:, P:C + 1], shh1)
        with tc.tile_pool(name="sym_ps", bufs=1, space="PSUM") as syp:
            tps = syp.tile([P, P], F32, name="tps", tag="tps")
            nc.tensor.transpose(tps, gx_sb[0][:, P:C],
                                ident_sb.bitcast(F32))
            nc.vector.tensor_copy(gx_sb[1][:, 0:P], tps)

    # Split the (large) diagonal out of Gx: products (Gx-D) w1 are ~100x
    # smaller, so the PE's per-product rounding no longer pollutes qk.
    # The diagonal term is applied exactly via per-partition multiplies.
    gxd = []
    for b in range(2):
        bs = slice(b * P, (b + 1) * P)
        dm = const.tile([P, P], F32, name=f"gxdm{b}", tag=f"gxdm{b}")
        nc.vector.tensor_mul(dm, gx_sb[b][:, bs], ident_sb.bitcast(F32))
        dcol = const.tile([P, 1], F32, name=f"gxd{b}", tag=f"gxd{b}")
        nc.vector.reduce_sum(out=dcol, in_=dm, axis=mybir.AxisListType.X)
        nc.vector.tensor_sub(gx_sb[b][:, bs], gx_sb[b][:, bs], dm)
        gxd.append(dcol)

    # --- tiny 256x256 algebra -------------------------------------------
    # All matrices in SBUF as two [128, *] row-blocks; vectors as [1, C] rows
    # or [128, 1] per-block columns.
    alg_sb = const  # persistent small tiles live in the const pool

    with tc.tile_pool(name="alg_ps", bufs=3, space="PSUM") as ap:
        # p_row = (A sx)^T, p3_row = (B sx)^T : lhsT = sx col (gx col 256)
        p_row = alg_sb.tile([1, C], F32, name="p_row", tag="p_row")
        p3_row = alg_sb.tile([1, C], F32, name="p3_row", tag="p3_row")
        for dst, wt in ((p_row, At), (p3_row, Bt)):
            vps = ap.tile([1, C], F32, name="vps", tag="algsmall", bufs=2)
            for k in range(2):
                nc.tensor.matmul(vps, gx_sb[k][:, C:C + 1], wt[k],
                                 start=(k == 0), stop=(k == 1))
            nc.vector.tensor_copy(dst, vps)

        # U' = (A (Gx-D))^T + exact D-correction : U'[j, o]
        u_sb = []
        for b in range(2):
            ups = ap.tile([P, C], F32, name="ups", tag="alg")
            for k in range(2):
                nc.tensor.matmul(ups, gx_sb[k][:, b * P:(b + 1) * P],
                                 At[k], start=(k == 0), stop=(k == 1))
            ud = alg_sb.tile([P, C], F32, name=f"u_d{b}", tag=f"u_d{b}")
            nc.vector.tensor_scalar_mul(ud, At[b], gxd[b])
            ut = alg_sb.tile([P, C], F32, name=f"u_sb{b}", tag=f"u_sb{b}")
            nc.vector.tensor_add(ut, ups, ud)
            u_sb.append(ut)

        # qk = U'^T B^T + p rho3^T + rho2 p3^T + N rho2 rho3^T ; softmax rows
        attn_sb = []
        for b in range(2):
            qkps = ap.tile([P, C], F32, name="qkps", tag="alg")
            for k in range(2):
                nc.tensor.matmul(qkps, u_sb[k][:, b * P:(b + 1) * P],
                                 Bt[k], start=(k == 0), stop=False)
            nc.tensor.matmul(qkps, p_row[:, b * P:(b + 1) * P],
                             rho3_row, start=False, stop=False)
            nc.tensor.matmul(qkps, rho2_row[:, b * P:(b + 1) * P],
                             p3_row, start=False, stop=False)
            nc.tensor.matmul(qkps, rho2_row[:, b * P:(b + 1) * P],
                             nrho3_row, start=False, stop=True)

            negmax = alg_sb.tile([P, 1], F32, name=f"negmax{b}", tag=f"nm{b}")
            nc.vector.tensor_reduce(
                out=negmax, in_=qkps, op=mybir.AluOpType.max,
                axis=mybir.AxisListType.X, negate=True,
            )
            expq = alg_sb.tile([P, C], F32, name=f"expq{b}", tag=f"expq{b}")
            nc.scalar.activation(
                out=expq, in_=qkps, func=mybir.ActivationFunctionType.Exp,
                bias=negmax, scale=1.0,
            )
            denom = alg_sb.tile([P, 1], F32, name=f"denom{b}", tag=f"dn{b}")
            nc.vector.reduce_sum(out=denom, in_=expq,
                                 axis=mybir.AxisListType.X)
            rden = alg_sb.tile([P, 1], F32, name=f"rden{b}", tag=f"rd{b}")
            nc.vector.reciprocal(rden, denom)
            at = alg_sb.tile([P, C], F32, name=f"attn{b}", tag=f"attn{b}")
            nc.vector.tensor_scalar_mul(at, expq, rden)
            attn_sb.append(at)

        # attn^T (4 PE transposes); fp32 copy for c0, fp16 copy for W
        attnT_sb = [
            alg_sb.tile([P, C], F32, name=f"attnT{j}", tag=f"attnT{j}")
            for j in range(2)
        ]
        attnT16 = [
            alg_sb.tile([P, C], F16, name=f"attnT16_{j}", tag=f"attnT16_{j}")
            for j in range(2)
        ]
        for b in range(2):
            for j in range(2):
                tps = ap.tile([P, P], F32, name="tps", tag="algtp", bufs=2)
                nc.tensor.transpose(tps,
                                    attn_sb[b][:, j * P:(j + 1) * P],
                                    ident_sb.bitcast(F32))
                nc.vector.tensor_copy(attnT_sb[j][:, b * P:(b + 1) * P], tps)
                nc.vector.tensor_copy(attnT16[j][:, b * P:(b + 1) * P], tps)

        # W^T = E-as-lhsT @ attn^T, all fp16 (W = attn E, E = w4 w1 from host)
        wt_sb = []
        for b in range(2):
            wps = ap.tile([P, C], F32, name="wps", tag="alg")
            for k in range(2):
                nc.tensor.matmul(wps, eh_sb[k][:, b * P:(b + 1) * P],
                                 attnT16[k],
                                 start=(k == 0), stop=(k == 1))
            wt_ = alg_sb.tile([P, C], F16, name=f"wt_sb{b}", tag=f"wt_sb{b}")
            nc.vector.tensor_copy(wt_, wps)
            wt_sb.append(wt_)

        # c0_col = attn (w4 b1 + b4) (per block; w4b from host)
        c0_col = []
        for b in range(2):
            cps = ap.tile([P, 1], F32, name="cps", tag="alg")
            for k in range(2):
                nc.tensor.matmul(cps,
                                 attnT_sb[k][:, b * P:(b + 1) * P].bitcast(F32),
                                 w4b_col[k].bitcast(F32), start=(k == 0),
                                 stop=(k == 1))
            ct = alg_sb.tile([P, 1], F32, name=f"c0_col{b}", tag=f"c0_col{b}")
            nc.vector.tensor_copy(ct, cps)
            c0_col.append(ct)

    # --- pass 2: out = W x + c0 1^T (all fp16 operands, fp16 output) -----
    # Fine-grained PSUM units ([128,512] x 8 bufs = all 8 banks) give the PE
    # enough runway to reach its top p-state; drains alternate scalar/vector
    # per unit; out-DMAs alternate queues per 1024-col staging tile.
    with tc.tile_pool(name="o_ps", bufs=8, space="PSUM") as ops, \
         tc.tile_pool(name="o_sb", bufs=4) as osb:
        nsub = 4
        SUBP = nsub * NT  # 2048 cols per staging tile
        n = 0
        for i in range(NPIECE):
            for b in range(2):
                for u in range(PIECE // SUBP):
                    ot = osb.tile([P, nsub, NT], F16, name="ot", tag="ot")
                    for tp in range(nsub // 2):
                        psts = [ops.tile([P, NT], F32, name="pst", tag="pst")
                                for _ in range(2)]
                        for k in range(2):
                            for j in range(2):
                                t = 2 * tp + j
                                nc.tensor.matmul(
                                    psts[j],
                                    wt_sb[k][:, b * P:(b + 1) * P],
                                    xs[k][i][:, u * SUBP + t * NT:
                                             u * SUBP + (t + 1) * NT],
                                    start=(k == 0),
                                    stop=(k == 1),
                                )
                        for j in range(2):
                            t = 2 * tp + j
                            if t % 2 == 0:
                                nc.scalar.activation(
                                    out=ot[:, t, :], in_=psts[j],
                                    func=mybir.ActivationFunctionType.Identity,
                                    bias=c0_col[b], scale=1.0,
                                )
                            else:
                                nc.vector.tensor_scalar_add(ot[:, t, :],
                                                            psts[j],
                                                            c0_col[b])
                    (nc.sync if n % 2 == 0 else nc.scalar).dma_start(
                        out=out_d[b * P:(b + 1) * P,
                                  i * PIECE + u * SUBP:
                                  i * PIECE + (u + 1) * SUBP],
                        in_=ot,
                    )
                    n += 1


def build_program(enable_asserts=False):
    nc = bacc.Bacc(
        "TRN2",
        target_bir_lowering=False,
        debug=False,
        enable_asserts=enable_asserts,
        num_devices=8,
    )
    d_in = {
        "x0": nc.dram_tensor("x0", [P, NPIX], F16, kind="ExternalInput").ap(),
        "x1": nc.dram_tensor("x1", [P, NPIX], F16, kind="ExternalInput").ap(),
        "wcat": nc.dram_tensor("wcat", [C, 2 * C], F32,
                               kind="ExternalInput").ap(),
        "brows": nc.dram_tensor("brows", [3, C], F32,
                                kind="ExternalInput").ap(),
        "bcols": nc.dram_tensor("bcols", [C, 4], F32,
                                kind="ExternalInput").ap(),
        "ident": nc.dram_tensor("ident", [P, P], F32R,
                                kind="ExternalInput").ap(),
        "identh": nc.dram_tensor("identh", [P, P], F16,
                                 kind="ExternalInput").ap(),
        "eh": nc.dram_tensor("eh", [C, C], F16, kind="ExternalInput").ap(),
    }
    d_out = {
        "out": nc.dram_tensor("out", [C, NPIX], F16,
                              kind="ExternalOutput").ap(),
    }
    with tile.TileContext(nc) as tc, ExitStack() as ctx:
        _emit(nc, tc, ctx, d_in, d_out)
    nc.compile()
    return nc


def make_in_maps(a, b, w1, b1, w2, b2, w3, b3, w4, b4):
    N = NPIX
    f = np.float32
    A = (w2 @ w1).astype(f)
    B = (w3 @ w1).astype(f)
    rho2 = (w2 @ b1 + b2).astype(f)
    rho3 = (w3 @ b1 + b3).astype(f)
    wcat = np.ascontiguousarray(np.concatenate(
        [A.T, B.T], axis=1).astype(f, copy=False))
    brows = np.ascontiguousarray(
        np.stack([rho2, rho3, N * rho3]).astype(f, copy=False))
    bcols = np.ascontiguousarray(
        np.stack([b1, N * b1, (w4 @ b1 + b4), np.ones(C, f)],
                 axis=1).astype(f))
    eh16 = np.ascontiguousarray((w4 @ w1).astype(np.float16))
    ident = np.eye(P, dtype=f)
    identh = np.eye(P, dtype=np.float16)
    Bsz = a.shape[0]
    in_maps = []
    for i in range(Bsz):
        in_maps.append({
            "x0": np.ascontiguousarray(a[i].reshape(P, N).astype(np.float16)),
            "x1": np.ascontiguousarray(b[i].reshape(P, N).astype(np.float16)),
            "wcat": wcat,
            "brows": brows,
            "bcols": bcols,
            "ident": ident,
            "identh": identh,
            "eh": eh16,
        })
    return in_maps


def _round_f32r(x):
    """Round fp32 to the FP32R-representable set (hi-bf16 + lo-bf16)."""
    import ml_dtypes

    x = np.asarray(x, np.float32)
    hi = x.astype(ml_dtypes.bfloat16).astype(np.float32)
    lo = (x - hi).astype(ml_dtypes.bfloat16).astype(np.float32)
    return np.ascontiguousarray(hi + lo)


_CACHE = {}


def kernel(a, b, w1, b1, w2, b2, w3, b3, w4, b4, _trace=False):
    a = np.asarray(a, dtype=np.float32)
    b = np.asarray(b, dtype=np.float32)
    args = [np.asarray(t, dtype=np.float32)
            for t in (w1, b1, w2, b2, w3, b3, w4, b4)]
    if "nc" not in _CACHE:
        _CACHE["nc"] = build_program()
    nc = _CACHE["nc"]
    in_maps = make_in_maps(a, b, *args)
    res = run_bass_kernel_spmd(nc, in_maps, core_ids=list(range(8)),
                               trace=_trace)
    B, Ch, H, W = a.shape
    out = np.stack([r["out"].astype(np.float32).reshape(C, H, W)
                    for r in res.results])
    if _trace:
        _CACHE["last_results"] = res
    return out


# revision 21
# speedup vs baseline: 1.0034x; 1.0034x over previous
"""Trainium2 Bass kernel for nn_CLF_block (channel-attention block).

Reference computation (per batch item b, with x = concat([a,b], ch) in [256, N],
N = H*W = 16384):
    z  = w1 x + b1 1^T
    q  = w2 z + b2 1^T ;  k = w3 z + b3 1^T ;  v = w4 z + b4 1^T
    qk = q k^T ; attn = softmax(qk, -1) ; out = attn v

Algebraic restructuring (two O(C^2 N) passes over x, everything else is
256x256 algebra):
    Gx = x x^T ; sx = x 1                      (one fp16 pass over x)
    u  = w1 sx ; s = u + N b1
    G  = w1 Gx w1^T + u b1^T + b1 u^T + N b1 b1^T        (= z z^T)
    qk = w2 G w3^T + (w2 s) b3^T + b2 (w3 s)^T + N b2 b3^T
    attn = softmax(qk)
    W  = attn (w4 w1) ; c0 = attn (w4 b1 + b4)  (w4 w1, w4 b1 + b4 from host)
    out = W x + c0 1^T                          (second fp16 pass over x)

x is downloaded ONCE as fp16 in natural layout (8 MiB/core); the gram pass
consumes on-chip PE transposes of it (Gx symmetry nearly halves the gram
work: block (1,0) is recovered as block (0,1)^T).  The 256x256 algebra runs
in fp32 on the PE with diagonal-splitting at the Gx and G levels (the same
scheme the reference two-download kernel used; hardware fp32r casts on the
DVE proved lossy, so no on-chip f32r is used).  The W = attn(w4 w1) product
runs in fp16.  Pass 2 is fp16 x fp16 with fine-grained PSUM units and the
output is written fp16 (8 MiB/core), upcast on the host.

HBM per core: ~9.8 MiB in + 8 MiB out (vs ~48 MiB for the two-download
scheme) -> ~118 us vs 217 us baseline.

Sharding: data-parallel over batch, one batch item per NeuronCore (B=8, 8 cores).
"""

import sys

if "/opt/trn_rl_repo" not in sys.path:
    sys.path.insert(0, "/opt/trn_rl_repo")

from contextlib import ExitStack

import numpy as np

import concourse.bass as bass
import concourse.mybir as mybir
import concourse.tile as tile
from concourse import bacc
from concourse.bass_utils import run_bass_kernel_spmd

F32 = mybir.dt.float32
F32R = mybir.dt.float32r
F16 = mybir.dt.float16
P = 128           # partitions / channel block
C = 256           # channels
NPIX = 128 * 128  # spatial positions per batch item
NPIECE = 4        # x DMA pieces per channel half
PIECE = NPIX // NPIECE   # 4096 cols per piece
NCHUNK = NPIX // P       # 128 gram chunks
GRP = 8           # chunks per transpose group (one PSUM bank = 8 x [128,128] fp16)
NGRP = NCHUNK // GRP     # 16 groups
NT = 512          # matmul moving-operand width for pass 2


def _emit(nc, tc, ctx, d_in, d_out):
    """Emit the Tile program for one core (one batch item)."""
    wcat, ident, identh = d_in["wcat"], d_in["ident"], d_in["identh"]
    x0_d, x1_d = d_in["x0"], d_in["x1"]
    brows, bcols = d_in["brows"], d_in["bcols"]
    out_d = d_out["out"]

    const = ctx.enter_context(tc.tile_pool(name="const", bufs=1))
    xpool = ctx.enter_context(tc.tile_pool(name="xpool", bufs=1))

    # --- small constants (head of the scalar queue, ~0.4 MiB) ------------
    identh_sb = const.tile([P, P], F16, name="identh_sb", tag="identh_sb")
    nc.sync.dma_start(out=identh_sb, in_=identh[:, :])

    rows = []
    for r in range(3):
        rt = const.tile([1, C], F32, name=f"brow{r}", tag=f"brow{r}")
        nc.scalar.dma_start(out=rt, in_=brows[r:r + 1, :])
        rows.append(rt)
    rho2_row, rho3_row, nrho3_row = rows

    bc_sb = []
    for k in range(2):
        bt = const.tile([P, 4], F32, name=f"bcol{k}", tag=f"bcol{k}")
        nc.scalar.dma_start(out=bt, in_=bcols[k * P:(k + 1) * P, :])
        bc_sb.append(bt)
    b1_col = [bc_sb[k][:, 0:1] for k in range(2)]
    nb1_col = [bc_sb[k][:, 1:2] for k in range(2)]
    w4b_col = [bc_sb[k][:, 2:3] for k in range(2)]

    eh = d_in["eh"]
    eh_sb = []
    for k in range(2):
        et = const.tile([P, C], F16, name=f"eh_sb{k}", tag=f"eh_sb{k}")
        nc.scalar.dma_start(out=et, in_=eh[k * P:(k + 1) * P, :])
        eh_sb.append(et)

    ident_sb = const.tile([P, P], F32R, name="ident_sb", tag="ident_sb")
    nc.scalar.dma_start(out=ident_sb, in_=ident[:, :])

    # --- resident fp16 x in natural layout.  All x pieces go on the sync
    # queue (scalar must stay free for phase-1 staging copies); the first
    # piece is split into 1024-col sub-DMAs so the PE can start early.
    xs = [[None] * NPIECE, [None] * NPIECE]
    for c in range(2):
        for i in range(NPIECE):
            xs[c][i] = xpool.tile([P, PIECE], F16, name=f"x{c}_{i}",
                                  tag=f"x{c}_{i}")
    SUB = 1024
    for s in range(PIECE // SUB):
        for c, src_d in ((0, x0_d), (1, x1_d)):
            nc.sync.dma_start(out=xs[c][0][:, s * SUB:(s + 1) * SUB],
                              in_=src_d[:, s * SUB:(s + 1) * SUB])
    for i in range(1, NPIECE):
        for c, src_d in ((0, x0_d), (1, x1_d)):
            nc.sync.dma_start(out=xs[c][i],
                              in_=src_d[:, i * PIECE:(i + 1) * PIECE])

    # --- big weights last on sync (needed only after the gram) -----------
    w_sb = []
    for k in range(2):
        wt = const.tile([P, 2 * C], F32, name=f"w_sb{k}", tag=f"w_sb{k}")
        nc.sync.dma_start(out=wt, in_=wcat[k * P:(k + 1) * P, :])
        w_sb.append(wt)
    At = [w_sb[k][:, 0 * C:1 * C] for k in range(2)]    # (w2 w1)^T  [c', o]
    Bt = [w_sb[k][:, 1 * C:2 * C] for k in range(2)]    # (w3 w1)^T  [c', o]

    # --- pass 1: Gx = x x^T via on-chip PE transposes --------------------
    # Per 128-col chunk: transpose both channel-half chunks into a PSUM
    # bank (8 chunks batched), copy to an ones-augmented fp16 staging tile,
    # then two accumulating gram matmuls.  Symmetry: only blocks (0,*) and
    # (1,1) are computed; block (1,0) = block (0,1)^T afterwards.
    stg = []
    for s in range(2):
        st = const.tile([P, GRP, C + 1], F16, name=f"stg{s}", tag=f"stg{s}")
        nc.vector.memset(st[:, :, C:C + 1], 1.0)
        stg.append(st)

    gx_sb = [
        const.tile([P, C + 1], F32, name=f"gx_sb{b}", tag=f"gx_sb{b}")
        for b in range(2)
    ]

    CH_PP = PIECE // P  # chunks per piece
    with tc.tile_pool(name="gx_ps", bufs=1, space="PSUM") as gxp, \
         tc.tile_pool(name="tp_ps", bufs=1, space="PSUM") as tpp:
        shh0 = gxp.tile([P, C + 1], F32, name="shh0", tag="shh0")
        shh1 = gxp.tile([P, P + 1], F32, name="shh1", tag="shh1")

        def emit_transposes(grp):
            s = grp % 2
            tpA = tpp.tile([P, GRP, P], F16, name="tpA", tag=f"tpA{s}")
            tpB = tpp.tile([P, GRP, P], F16, name="tpB", tag=f"tpB{s}")
            for g in range(GRP):
                ch = grp * GRP + g
                i, lc = divmod(ch, CH_PP)
                nc.tensor.transpose(tpA[:, g, :],
                                    xs[0][i][:, lc * P:(lc + 1) * P],
                                    identh_sb)
                nc.tensor.transpose(tpB[:, g, :],
                                    xs[1][i][:, lc * P:(lc + 1) * P],
                                    identh_sb)
            nc.vector.tensor_copy(stg[s][:, :, 0:P], tpA)
            nc.scalar.activation(out=stg[s][:, :, P:C], in_=tpB,
                                 func=mybir.ActivationFunctionType.Identity,
                                 scale=1.0)

        def emit_grams(grp):
            s = grp % 2
            for g in range(GRP):
                ch = grp * GRP + g
                nc.tensor.matmul(shh0, stg[s][:, g, 0:P],
                                 stg[s][:, g, 0:C + 1],
                                 start=(ch == 0), stop=(ch == NCHUNK - 1))
                nc.tensor.matmul(shh1, stg[s][:, g, P:C],
                                 stg[s][:, g, P:C + 1],
                                 start=(ch == 0), stop=(ch == NCHUNK - 1))

        for grp in range(NGRP + 1):
            if grp < NGRP:
                emit_transposes(grp)
            if grp >= 1:
                emit_grams(grp - 1)

        # Assemble full Gx (with sx in col 256) from the symmetric pieces.
        nc.vector.tensor_copy(gx_sb[0], shh0)
        nc.vector.tensor_copy(gx_sb[1][:, P:C + 1], shh1)
        with tc.tile_pool(name="sym_ps", bufs=1, space="PSUM") as syp:
            tps = syp.tile([P, P], F32, name="tps", tag="tps")
            nc.tensor.transpose(tps, gx_sb[0][:, P:C],
                                ident_sb.bitcast(F32))
            nc.vector.tensor_copy(gx_sb[1][:, 0:P], tps)

    # Split the (large) diagonal out of Gx: products (Gx-D) w1 are ~100x
    # smaller, so the PE's per-product rounding no longer pollutes qk.
    # The diagonal term is applied exactly via per-partition multiplies.
    gxd = []
    for b in range(2):
        bs = slice(b * P, (b + 1) * P)
        dm = const.tile([P, P], F32, name=f"gxdm{b}", tag=f"gxdm{b}")
        nc.vector.tensor_mul(dm, gx_sb[b][:, bs], ident_sb.bitcast(F32))
        dcol = const.tile([P, 1], F32, name=f"gxd{b}", tag=f"gxd{b}")
        nc.vector.reduce_sum(out=dcol, in_=dm, axis=mybir.AxisListType.X)
        nc.vector.tensor_sub(gx_sb[b][:, bs], gx_sb[b][:, bs], dm)
        gxd.append(dcol)

    # --- tiny 256x256 algebra -------------------------------------------
    # All matrices in SBUF as two [128, *] row-blocks; vectors as [1, C] rows
    # or [128, 1] per-block columns.
    alg_sb = const  # persistent small tiles live in the const pool

    with tc.tile_pool(name="alg_ps", bufs=3, space="PSUM") as ap:
        # p_row = (A sx)^T, p3_row = (B sx)^T : lhsT = sx col (gx col 256)
        p_row = alg_sb.tile([1, C], F32, name="p_row", tag="p_row")
        p3_row = alg_sb.tile([1, C], F32, name="p3_row", tag="p3_row")
        for dst, wt in ((p_row, At), (p3_row, Bt)):
            vps = ap.tile([1, C], F32, name="vps", tag="algsmall", bufs=2)
            for k in range(2):
                nc.tensor.matmul(vps, gx_sb[k][:, C:C + 1], wt[k],
                                 start=(k == 0), stop=(k == 1))
            nc.vector.tensor_copy(dst, vps)

        # U' = (A (Gx-D))^T + exact D-correction : U'[j, o]
        u_sb = []
        for b in range(2):
            ups = ap.tile([P, C], F32, name="ups", tag="alg")
            for k in range(2):
                nc.tensor.matmul(ups, gx_sb[k][:, b * P:(b + 1) * P],
                                 At[k], start=(k == 0), stop=(k == 1))
            ud = alg_sb.tile([P, C], F32, name=f"u_d{b}", tag=f"u_d{b}")
            nc.vector.tensor_scalar_mul(ud, At[b], gxd[b])
            ut = alg_sb.tile([P, C], F32, name=f"u_sb{b}", tag=f"u_sb{b}")
            nc.vector.tensor_add(ut, ups, ud)
            u_sb.append(ut)

        # qk = U'^T B^T + p rho3^T + rho2 p3^T + N rho2 rho3^T ; softmax rows
        attn_sb = []
        for b in range(2):
            qkps = ap.tile([P, C], F32, name="qkps", tag="alg")
            for k in range(2):
                nc.tensor.matmul(qkps, u_sb[k][:, b * P:(b + 1) * P],
                                 Bt[k], start=(k == 0), stop=False)
            nc.tensor.matmul(qkps, p_row[:, b * P:(b + 1) * P],
                             rho3_row, start=False, stop=False)
            nc.tensor.matmul(qkps, rho2_row[:, b * P:(b + 1) * P],
                             p3_row, start=False, stop=False)
            nc.tensor.matmul(qkps, rho2_row[:, b * P:(b + 1) * P],
                             nrho3_row, start=False, stop=True)

            negmax = alg_sb.tile([P, 1], F32, name=f"negmax{b}", tag=f"nm{b}")
            nc.vector.tensor_reduce(
                out=negmax, in_=qkps, op=mybir.AluOpType.max,
                axis=mybir.AxisListType.X, negate=True,
            )
            expq = alg_sb.tile([P, C], F32, name=f"expq{b}", tag=f"expq{b}")
            nc.scalar.activation(
                out=expq, in_=qkps, func=mybir.ActivationFunctionType.Exp,
                bias=negmax, scale=1.0,
            )
            denom = alg_sb.tile([P, 1], F32, name=f"denom{b}", tag=f"dn{b}")
            nc.vector.reduce_sum(out=denom, in_=expq,
                                 axis=mybir.AxisListType.X)
            rden = alg_sb.tile([P, 1], F32, name=f"rden{b}", tag=f"rd{b}")
            nc.vector.reciprocal(rden, denom)
            at = alg_sb.tile([P, C], F32, name=f"attn{b}", tag=f"attn{b}")
            nc.vector.tensor_scalar_mul(at, expq, rden)
            attn_sb.append(at)

        # attn^T (4 PE transposes); fp32 copy for c0, fp16 copy for W
        attnT_sb = [
            alg_sb.tile([P, C], F32, name=f"attnT{j}", tag=f"attnT{j}")
            for j in range(2)
        ]
        attnT16 = [
            alg_sb.tile([P, C], F16, name=f"attnT16_{j}", tag=f"attnT16_{j}")
            for j in range(2)
        ]
        for b in range(2):
            for j in range(2):
                tps = ap.tile([P, P], F32, name="tps", tag="algtp", bufs=2)
                nc.tensor.transpose(tps,
                                    attn_sb[b][:, j * P:(j + 1) * P],
                                    ident_sb.bitcast(F32))
                nc.vector.tensor_copy(attnT_sb[j][:, b * P:(b + 1) * P], tps)
                nc.vector.tensor_copy(attnT16[j][:, b * P:(b + 1) * P], tps)

        # W^T = E-as-lhsT @ attn^T, all fp16 (W = attn E, E = w4 w1 from host)
        wt_sb = []
        for b in range(2):
            wps = ap.tile([P, C], F32, name="wps", tag="alg")
            for k in range(2):
                nc.tensor.matmul(wps, eh_sb[k][:, b * P:(b + 1) * P],
                                 attnT16[k],
                                 start=(k == 0), stop=(k == 1))
            wt_ = alg_sb.tile([P, C], F16, name=f"wt_sb{b}", tag=f"wt_sb{b}")
            nc.vector.tensor_copy(wt_, wps)
            wt_sb.append(wt_)

        # c0_col = attn (w4 b1 + b4) (per block; w4b from host)
        c0_col = []
        for b in range(2):
            cps = ap.tile([P, 1], F32, name="cps", tag="alg")
            for k in range(2):
                nc.tensor.matmul(cps,
                                 attnT_sb[k][:, b * P:(b + 1) * P].bitcast(F32),
                                 w4b_col[k].bitcast(F32), start=(k == 0),
                                 stop=(k == 1))
            ct = alg_sb.tile([P, 1], F32, name=f"c0_col{b}", tag=f"c0_col{b}")
            nc.vector.tensor_copy(ct, cps)
            c0_col.append(ct)

    # --- pass 2: out = W x + c0 1^T (all fp16 operands, fp16 output) -----
    # Fine-grained PSUM units ([128,512] x 8 bufs = all 8 banks) give the PE
    # enough runway to reach its top p-state; drains alternate scalar/vector
    # per unit; out-DMAs alternate queues per 1024-col staging tile.
    with tc.tile_pool(name="o_ps", bufs=8, space="PSUM") as ops, \
         tc.tile_pool(name="o_sb", bufs=4) as osb:
        nsub = 4
        SUBP = nsub * NT  # 2048 cols per staging tile
        n = 0
        for i in range(NPIECE):
            for b in range(2):
                for u in range(PIECE // SUBP):
                    ot = osb.tile([P, nsub, NT], F16, name="ot", tag="ot")
                    for t in range(nsub):
                        pst = ops.tile([P, NT], F32, name="pst", tag="pst")
                        for k in range(2):
                            nc.tensor.matmul(
                                pst,
                                wt_sb[k][:, b * P:(b + 1) * P],
                                xs[k][i][:, u * SUBP + t * NT:
                                         u * SUBP + (t + 1) * NT],
                                start=(k == 0),
                                stop=(k == 1),
                            )
                        if (2 * n + t) % 2 == 0:
                            nc.scalar.activation(
                                out=ot[:, t, :], in_=pst,
                                func=mybir.ActivationFunctionType.Identity,
                                bias=c0_col[b], scale=1.0,
                            )
                        else:
                            nc.vector.tensor_scalar_add(ot[:, t, :], pst,
                                                        c0_col[b])
                    (nc.sync if n % 2 == 0 else nc.scalar).dma_start(
                        out=out_d[b * P:(b + 1) * P,
                                  i * PIECE + u * SUBP:
                                  i * PIECE + (u + 1) * SUBP],
                        in_=ot,
                    )
                    n += 1


def build_program(enable_asserts=False):
    nc = bacc.Bacc(
        "TRN2",
        target_bir_lowering=False,
        debug=False,
        enable_asserts=enable_asserts,
        num_devices=8,
    )
    d_in = {
        "x0": nc.dram_tensor("x0", [P, NPIX], F16, kind="ExternalInput").ap(),
        "x1": nc.dram_tensor("x1", [P, NPIX], F16, kind="ExternalInput").ap(),
        "wcat": nc.dram_tensor("wcat", [C, 2 * C], F32,
                               kind="ExternalInput").ap(),
        "brows": nc.dram_tensor("brows", [3, C], F32,
                                kind="ExternalInput").ap(),
        "bcols": nc.dram_tensor("bcols", [C, 4], F32,
                                kind="ExternalInput").ap(),
        "ident": nc.dram_tensor("ident", [P, P], F32R,
                                kind="ExternalInput").ap(),
        "identh": nc.dram_tensor("identh", [P, P], F16,
                                 kind="ExternalInput").ap(),
        "eh": nc.dram_tensor("eh", [C, C], F16, kind="ExternalInput").ap(),
    }
    d_out = {
        "out": nc.dram_tensor("out", [C, NPIX], F16,
                              kind="ExternalOutput").ap(),
    }
    with tile.TileContext(nc) as tc, ExitStack() as ctx:
        _emit(nc, tc, ctx, d_in, d_out)
    nc.compile()
    return nc


def make_in_maps(a, b, w1, b1, w2, b2, w3, b3, w4, b4):
    N = NPIX
    f = np.float32
    A = (w2 @ w1).astype(f)
    B = (w3 @ w1).astype(f)
    rho2 = (w2 @ b1 + b2).astype(f)
    rho3 = (w3 @ b1 + b3).astype(f)
    wcat = np.ascontiguousarray(np.concatenate(
        [A.T, B.T], axis=1).astype(f, copy=False))
    brows = np.ascontiguousarray(
        np.stack([rho2, rho3, N * rho3]).astype(f, copy=False))
    bcols = np.ascontiguousarray(
        np.stack([b1, N * b1, (w4 @ b1 + b4), np.ones(C, f)],
                 axis=1).astype(f))
    eh16 = np.ascontiguousarray((w4 @ w1).astype(np.float16))
    ident = np.eye(P, dtype=f)
    identh = np.eye(P, dtype=np.float16)
    Bsz = a.shape[0]
    in_maps = []
    for i in range(Bsz):
        in_maps.append({
            "x0": np.ascontiguousarray(a[i].reshape(P, N).astype(np.float16)),
            "x1": np.ascontiguousarray(b[i].reshape(P, N).astype(np.float16)),
            "wcat": wcat,
            "brows": brows,
            "bcols": bcols,
            "ident": ident,
            "identh": identh,
            "eh": eh16,
        })
    return in_maps


def _round_f32r(x):
    """Round fp32 to the FP32R-representable set (hi-bf16 + lo-bf16)."""
    import ml_dtypes

    x = np.asarray(x, np.float32)
    hi = x.astype(ml_dtypes.bfloat16).astype(np.float32)
    lo = (x - hi).astype(ml_dtypes.bfloat16).astype(np.float32)
    return np.ascontiguousarray(hi + lo)


_CACHE = {}


def kernel(a, b, w1, b1, w2, b2, w3, b3, w4, b4, _trace=False):
    a = np.asarray(a, dtype=np.float32)
    b = np.asarray(b, dtype=np.float32)
    args = [np.asarray(t, dtype=np.float32)
            for t in (w1, b1, w2, b2, w3, b3, w4, b4)]
    if "nc" not in _CACHE:
        _CACHE["nc"] = build_program()
    nc = _CACHE["nc"]
    in_maps = make_in_maps(a, b, *args)
    res = run_bass_kernel_spmd(nc, in_maps, core_ids=list(range(8)),
                               trace=_trace)
    B, Ch, H, W = a.shape
    out = np.stack([r["out"].astype(np.float32).reshape(C, H, W)
                    for r in res.results])
    if _trace:
        _CACHE["last_results"] = res
    return out
